# revision 5
# baseline (speedup 1.0000x reference)
"""Distributed GPT-2 causal attention block for 8 TRN2 NeuronCores.

Sharding: data-parallel over batch (B=2) x tensor-parallel over heads
(16 heads -> 4 groups of 4). core = b*4 + g handles batch b, heads 4g..4g+3.

Per-core kernel (all compute in bf16, f32 PSUM accumulation):
  qT/kT = W[q|k]^T x^T        [2 tiles of 128 = 2 heads each, layout (h d) x S]
  v     = x W_v               [S x (4 heads x 65)], augmented with a ones
          column per head
  sT    = kT^T qT (transposed scores), emitted as 512-query CHUNK-PAIRS:
          head hl0 chunk at psum col 0 (PE row tile (0,0)) then hl1 chunk
          at col 512 (row tile (64,0)) in one [128,1024] sps slot --
          adjacent matmuls always hit opposite PE row tiles, which the HW
          streams CONCURRENTLY (~2x, trace-verified ~300ns overlap on
          ~370ns instructions)
  PT    = exp(sT/8): ONE strided ScalarE activation per chunk-pair (2-bank
          3D src AP) into a [128, 2w] tile (halves the ACT count -> 256ns
          fixed cost each); causal diag chunk masked multiplicatively
          (VectorE), blocks above diag never computed
  av    = v_aug^T PT accumulated over key blocks (full 128-row matmuls,
          serial -- see row-tile note below); r rides as row 64
  rinv  = pair-0: r DRAM roundtrip broadcast; pair-1: K=1 ones-matmul
          broadcast back into the av slot, head A's full chain before head
          B's so avA's bank recycles early
  outT  = wp^T attT partial (sum over head groups on host); proj cols
          0:1536 woven into pair-1 (drains on VectorE, off the exp
          stream); cols 1536:2048 as 256-wide units, [1536:1792] woven
          into the last (256-wide) pair-1 segments, final units alternate
          Scalar/Vector copies + sync/gpsimd DMA queues

Host: shard/cast inputs, run SPMD on cores 0-7, transpose+reduce partials,
fold b_attn's v-bias and b_proj in on the host (exact: softmax rows sum to 1).

Perf state: ~164.4us paired-median (v10; v6 was ~169, baseline ~178
same-tier; harness baseline 200044). v10's win: pair-1's FIRST segment
(AV + tail) rides ONE long-held sps slot inside run_pair(0)'s weave --
the exp-bound middle has ~16us of PE idle that absorbs it -- and the
proj weave starts at gstep 8. Won 4/4 paired reps, -4.8us. DVFS drifts +-6-16%: ONLY trust ab_test.py (alternates
compiled variants in one process). Breakdown at 168us: ~10.5 startup (6.5
fixed framework preamble incl TWO 8-core barriers - runtime-imposed,
num_devices=1 does NOT remove them; ~4 DMA ramp), matmul union ~132 (sum
234 w/ score-pair overlap), ~7.5 fixed teardown, ~16 mid-kernel PE gaps.
ScalarE exp stream ~85us.

HW limits verified (micro_test*.py): PE row-band-64 matmuls (tile_position
(64,0)) CANNOT psum-accumulate: start=False into a (0,0)-opened group =>
device fault; onto its own (64,0) group => silently doesn't add. (0,0)
64-row accumulate works. So AV cannot be row-split for 2x -- single-shot
tiled matmuls only (scores). Also: matmul N<=512 and must not cross a
PSUM bank; tensor_tensor cannot read TWO psum operands (walrus verifier);
fp8 AV breaks the 2e-2 budget (~4-5%: softmax weights are O(1) random-sign
sums, per-element fp8 error doesn't average away); PSUM base partition in
{0,32,64}; no DMA from PSUM.

REJECTED via paired A/B: r-broadcast into a psp slice (steals a proj-weave
slot, +6us); num_devices=1 (barriers come from the runtime); plus the
prior session's list: masks/elementwise on GpSimd, drains on ScalarE
mid-kernel, merged pair schedules, half-width qk waves, dual-engine x
loads. Remaining theoretical floor ~120-125us for this dataflow (qkv+proj
~57us serial 128-row, av ~29 serial, scores ~15 overlapped, edges ~18
fixed); closing more needs fewer matmul columns, which bf16 + this
sharding does not allow.
"""

import numpy as np
import ml_dtypes

B, S, NX = 2, 2048, 1024
H, D = 16, 64
HPC = 4        # heads per core
KCH = NX // 128  # 8 contraction chunks
SQT = S // 128   # 16 query tiles
SCALE = 0.125    # 1/sqrt(D)

_nc_cache = None


def _sub512(lo, hi):
    """split [lo,hi) on the 512 grid."""
    out = []
    s = lo
    while s < hi:
        e = min((s // 512 + 1) * 512, hi)
        out.append((s, e))
        s = e
    return out


def _emit(nc, tc, bass, mybir, tens):
    dt = mybir.dt
    F32, BF16 = dt.float32, dt.bfloat16
    MULT = mybir.AluOpType.mult
    EXP = mybir.ActivationFunctionType.Exp
    xT, wqk, wv, wp, bq, bk, maskT, outT, ri_dram = tens

    import contextlib
    with contextlib.ExitStack() as ctx:
        consts = ctx.enter_context(tc.tile_pool(name="consts", bufs=1))
        wpool = ctx.enter_context(tc.tile_pool(name="wpool", bufs=1))
        xpool = ctx.enter_context(tc.tile_pool(name="xpool", bufs=1))
        qkpool = ctx.enter_context(tc.tile_pool(name="qkpool", bufs=1))
        vpool = ctx.enter_context(tc.tile_pool(name="vpool", bufs=1))
        ptpool = ctx.enter_context(tc.tile_pool(name="ptpool", bufs=1))
        atpool = ctx.enter_context(tc.tile_pool(name="atpool", bufs=1))
        rpool = ctx.enter_context(tc.tile_pool(name="rpool", bufs=3))
        opool = ctx.enter_context(tc.tile_pool(name="opool", bufs=6))
        psp = ctx.enter_context(tc.tile_pool(name="psp", bufs=3, space="PSUM"))
        avp = ctx.enter_context(tc.tile_pool(name="avp", bufs=2, space="PSUM"))

        # ---- weights + inputs, split per k-chunk and interleaved so the
        # first qk wave can start after ~2 chunks instead of the full load
        wqk_sb = wpool.tile([128, KCH, 2 * HPC * D], BF16, tag="wqk")
        x_sb = [xpool.tile([128, S], BF16, tag=f"x{k}", name=f"x{k}")
                for k in range(KCH)]
        # issue input DMAs from two engines in parallel (Sync issue is
        # ~600ns per dma_start; serial issue of all of them costs ~10us)
        for k in range(KCH):
            if k == 0:
                # first chunks gate the whole pipeline: split by COLUMNS so
                # the first wave's operands (wq cols 0:128, x cols 0:1024)
                # land before the rest of the transfer
                nc.sync.dma_start(out=wqk_sb[:, k, 0:128],
                                  in_=wqk.ap()[0:128, 0:128])
                nc.sync.dma_start(out=x_sb[k][:, 0:512],
                                  in_=xT.ap()[0:128, 0:512])
                nc.sync.dma_start(out=x_sb[k][:, 512:1024],
                                  in_=xT.ap()[0:128, 512:1024])
                nc.sync.dma_start(out=x_sb[k][:, 1024:2048],
                                  in_=xT.ap()[0:128, 1024:2048])
                nc.sync.dma_start(out=wqk_sb[:, k, 128:512],
                                  in_=wqk.ap()[0:128, 128:512])
            else:
                nc.sync.dma_start(out=wqk_sb[:, k, :],
                                  in_=wqk.ap()[k * 128:(k + 1) * 128, :])
                nc.sync.dma_start(out=x_sb[k][:],
                                  in_=xT.ap()[k * 128:(k + 1) * 128, :])
        wq_sb = wqk_sb.rearrange("p k (w n) -> p k w n", w=2)[:, :, 0, :]
        wk_sb = wqk_sb.rearrange("p k (w n) -> p k w n", w=2)[:, :, 1, :]
        bq_sb = consts.tile([128, 2], F32, tag="bq")
        nc.sync.dma_start(out=bq_sb[:, :],
                          in_=bq.ap().rearrange("(t p) o -> p (t o)", p=128))
        bk_sb = consts.tile([128, 2], F32, tag="bk")
        nc.sync.dma_start(out=bk_sb[:, :],
                          in_=bk.ap().rearrange("(t p) o -> p (t o)", p=128))
        mask_sb = consts.tile([128, 128], BF16, tag="mask")
        nc.sync.dma_start(out=mask_sb[:, :], in_=maskT.ap())
        ones_sb = consts.tile([65, 64], BF16, tag="ones")
        nc.gpsimd.memset(ones_sb[64:65, :], 1.0)
        # wv/wp are not needed until the v units / proj weave: issue their
        # (bulky) DMAs after every x/wqk chunk so the chunk cadence that
        # gates the qk ramp stays tight
        wv_sb = wpool.tile([128, KCH, HPC * D], BF16, tag="wv")
        nc.sync.dma_start(out=wv_sb[:],
                          in_=wv.ap().rearrange("(k p) n -> p k n", p=128))
        wp_sb = wpool.tile([128, 2, NX], BF16, tag="wp")
        nc.sync.dma_start(out=wp_sb[:],
                          in_=wp.ap().rearrange("(k p) n -> p k n", p=128))

        qt_sb, kt_sb = [], []
        for t in range(2):
            qt_sb.append(qkpool.tile([128, S], BF16, tag=f"qt{t}", name=f"qt{t}"))
            kt_sb.append(qkpool.tile([128, S], BF16, tag=f"kt{t}", name=f"kt{t}"))
        v_sb = [vpool.tile([128, HPC * 65], BF16, tag=f"v{j}", name=f"v{j}")
                for j in range(SQT)]
        attT = [atpool.tile([128, S], BF16, tag=f"attT{hp}", name=f"attT{hp}")
                for hp in range(2)]

        def emit_qk_wave_steps(t, which, cpair=None):
            # one wave: sq-chunks cpair of qT (which=0) or kT (which=1) of
            # pair t, k-outer; holds ONE av-tag psum slot.
            (dst, wsb, bsb) = ((qt_sb, wq_sb, bq_sb), (kt_sb, wk_sb, bk_sb))[which]
            if cpair is None:
                return (emit_qk_wave_steps(t, which, (0, 1)) +
                        emit_qk_wave_steps(t, which, (2, 3)))
            ps = psp.tile([128, 1024], F32, tag="sps",
                          name=f"qkw{t}{which}{cpair[0]}")
            c0 = cpair[0] * 512
            def kstep(k):
                for ci, c in enumerate(cpair):
                    nc.tensor.matmul(
                        ps[:, ci * 512:(ci + 1) * 512],
                        lhsT=wsb[:, k, t * 128:(t + 1) * 128],
                        rhs=x_sb[k][:, c * 512:(c + 1) * 512],
                        start=(k == 0), stop=(k == KCH - 1))
            def drain():
                nc.vector.tensor_scalar_add(
                    out=dst[t][:, c0:c0 + 1024],
                    in0=ps[:, 0:1024],
                    scalar1=bsb[:, t:t + 1])
            steps = [lambda k=k: kstep(k) for k in range(KCH)]
            steps.append(drain)
            return steps

        def emit_v_unit(j):
            vt3 = v_sb[j].rearrange("p (h e) -> p h e", e=65)
            nc.gpsimd.memset(vt3[:, :, 64:65], 1.0)
            ps = psp.tile([128, 512], F32, tag="sps")
            for k in range(KCH):
                nc.tensor.matmul(
                    ps[:, 0:HPC * D],
                    lhsT=x_sb[k][:, j * 128:(j + 1) * 128],
                    rhs=wv_sb[:, k, :],
                    start=(k == 0), stop=(k == KCH - 1))
            nc.vector.tensor_copy(
                out=vt3[:, :, 0:64],
                in_=ps[:, 0:HPC * D].rearrange("p (h d) -> p h d", d=64))

        # PT chunk-pair tiles, keyed (p, j, half, cell) where cell is the
        # 512-grid cell index; each tile holds BOTH heads' exp-scores for
        # one <=512-wide query chunk: cols [0:w] = head hl0, [w:2w] = hl1.
        # tags shared across pairs
        pt = {}

        def emit_score_chunk(p, j, half, a, b):
            """both heads of pair p for key-block j, query chunk [a,b):
            h0 chunk (PE row tile T0) then h1 chunk (T8) into one sps slot
            -- adjacent matmuls always hit opposite PE row tiles so they
            stream concurrently -- then ONE merged exp."""
            w = b - a
            cell = a // 512
            pt_t = ptpool.tile([128, 2 * w], BF16,
                               tag=f"pt_{j}_{half}_{cell}",
                               name=f"pt_{j}_{half}_{cell}_p{p}")
            pt[(p, j, half, cell)] = (pt_t, a, w)
            ps = psp.tile([128, 1024], F32, tag="sps")
            for hl in range(2):
                # hl1 at column 512 (second PSUM bank): a matmul output may
                # not cross a bank boundary
                nc.tensor.matmul(
                    ps[:, hl * 512:hl * 512 + w],
                    lhsT=kt_sb[p][hl * 64:(hl + 1) * 64,
                                  j * 128:(j + 1) * 128],
                    rhs=qt_sb[p][hl * 64:(hl + 1) * 64, a:b],
                    start=True, stop=True)
            nc.scalar.activation(
                out=pt_t.rearrange("p (u c) -> p u c", u=2),
                in_=ps.rearrange("p (u c) -> p u c", u=2)[:, :, 0:w],
                func=EXP, scale=SCALE)
            if a <= j * 128 < b:
                o = j * 128 - a
                for hl in range(2):
                    nc.vector.tensor_tensor(
                        out=pt_t[:, hl * w + o:hl * w + o + 128],
                        in0=pt_t[:, hl * w + o:hl * w + o + 128],
                        in1=mask_sb[:, :], op=MULT)

        def emit_av(p, hl, base, width, j, av):
            h = p * 2 + hl
            half = 0 if base < 1024 else 1
            lo = max(j * 128, base)
            for (a, b) in _sub512(lo, base + width):
                pt_t, ca, cw = pt[(p, j, half, a // 512)]
                nc.tensor.matmul(
                    av[:, a - base:b - base],
                    lhsT=v_sb[j][:, h * 65:(h + 1) * 65],
                    rhs=pt_t[:, hl * cw + a - ca:hl * cw + b - ca],
                    start=(j == 0), stop=(j == (base + width) // 128 - 1),
                    skip_group_check=True)

        def emit_tail(p, base, avA, avB, width=512):
            # One copy per head drains av[0:65] out of PSUM immediately
            # (r rides along as row 64) so the next segment's AV matmuls can
            # recycle the PSUM slot without waiting on the r broadcast.
            w = width
            rawA = rpool.tile([65, 512], BF16, tag="rawA", name=f"rawA{p}_{base}")
            rawB = rpool.tile([65, 512], BF16, tag="rawB", name=f"rawB{p}_{base}")
            nc.vector.tensor_copy(out=rawA[:, 0:w], in_=avA[0:65, 0:w])
            nc.vector.tensor_copy(out=rawB[:, 0:w], in_=avB[0:65, 0:w])
            gs = slice(base, base + w)
            tb = rpool.tile([64, 512], BF16, tag="tb", name=f"tb{p}_{base}")
            if p == 1:
                # pair-1 tails gate the proj weave: broadcast r across
                # partitions with a K=1 ones-matmul back into the (just
                # drained) av PSUM -- ~3us less latency than the DRAM
                # roundtrip, on an otherwise idle TensorE window.
                # complete head A's bcast/recip/normalize chain BEFORE
                # starting head B's so avA's PSUM slot frees ~1us earlier
                # for the next segment's AV matmuls
                nc.tensor.matmul(
                    avA[0:64, 0:w], lhsT=ones_sb[64:65, :],
                    rhs=rawA[64:65, 0:w], start=True, stop=True,
                    skip_group_check=True)
                nc.vector.reciprocal_approx_fast(out=avA[0:64, 0:w],
                                                 in_=avA[0:64, 0:w])
                nc.vector.tensor_tensor(
                    out=attT[p][0:64, gs], in0=rawA[0:64, 0:w],
                    in1=avA[0:64, 0:w], op=MULT)
                nc.tensor.matmul(
                    avB[0:64, 0:w], lhsT=ones_sb[64:65, :],
                    rhs=rawB[64:65, 0:w], start=True, stop=True,
                    skip_group_check=True)
                nc.vector.reciprocal_approx_fast(out=avB[0:64, 0:w],
                                                 in_=avB[0:64, 0:w])
                nc.vector.tensor_tensor(
                    out=tb[:, 0:w], in0=rawB[0:64, 0:w], in1=avB[0:64, 0:w],
                    op=MULT)
            else:
                nc.sync.dma_start(out=ri_dram.ap()[2 * p:2 * p + 1,
                                                   base:base + w],
                                  in_=rawA[64:65, 0:w])
                nc.sync.dma_start(out=ri_dram.ap()[2 * p + 1:2 * p + 2,
                                                   base:base + w],
                                  in_=rawB[64:65, 0:w])
                rt = rpool.tile([64, 1024], F32, tag="rt", name=f"rt{p}_{base}")
                nc.gpsimd.dma_start(
                    out=rt[:, 0:2 * w],
                    in_=bass.AP(tensor=ri_dram, offset=2 * p * S + base,
                                ap=[[0, 64], [S, 2], [1, w]]))
                nc.vector.reciprocal_approx_fast(out=rt[:, 0:2 * w],
                                                 in_=rt[:, 0:2 * w])
                nc.vector.tensor_tensor(
                    out=attT[p][0:64, gs], in0=rawA[0:64, 0:w],
                    in1=rt[:, 0:w], op=MULT)
                nc.vector.tensor_tensor(
                    out=tb[:, 0:w], in0=rawB[0:64, 0:w], in1=rt[:, w:2 * w],
                    op=MULT)
            # pair-1's attT write gates the proj weave; gpsimd's DMA queue
            # is empty there while Sync holds cast-gated output DMAs
            (nc.gpsimd if p == 1 else nc.sync).dma_start(
                out=attT[p][64:128, gs], in_=tb[:, 0:w])

        # ================= emission =================
        def proj_unit(cl, n, wid=512):
            ps = psp.tile([128, wid], F32, tag="sps", name=f"pps{cl}_{n}")
            for (a, b) in _sub512(0, wid):
                for kk in range(2):
                    nc.tensor.matmul(
                        ps[:, a:b],
                        lhsT=wp_sb[:, kk, n * 128:(n + 1) * 128],
                        rhs=attT[kk][:, cl + a:cl + b],
                        start=(kk == 0), stop=(kk == 1))
            osb = opool.tile([128, wid], BF16, tag="osb", name=f"osb{cl}_{n}")
            if cl >= 1536:
                # end phase: exp stream is done; alternate ScalarE/VectorE
                # copies and the two DMA queues so the 16 final units don't
                # serialize on one engine each
                (nc.scalar.copy if n % 2 == 0 else
                 (lambda out, in_: nc.vector.tensor_copy(out=out, in_=in_)))(
                    out=osb[:, :], in_=ps[:, :])
                (nc.gpsimd if cl >= 1792 else nc.sync).dma_start(
                    out=outT.ap()[n * 128:(n + 1) * 128, cl:cl + wid],
                    in_=osb[:, :])
            else:
                # mid-kernel: keep PSUM drains off the saturated exp stream
                nc.vector.tensor_copy(out=osb[:, :], in_=ps[:, :])
                nc.sync.dma_start(
                    out=outT.ap()[n * 128:(n + 1) * 128, cl:cl + wid],
                    in_=osb[:, :])

        # qk pair 0 rides the input-DMA ramp (k-outer waves): interleave
        # THREE chains (q chains A,B + kt chain C = all 3 psp slots) k-major
        # so every arriving x-chunk DMA unleashes ~3072 matmul columns --
        # the DMA feed is the ramp bottleneck, chain-major would idle the
        # PE between chunk arrivals. Chain D then reuses the loaded chunks
        # at full rate.
        qsteps = emit_qk_wave_steps(0, 0)
        ksteps = emit_qk_wave_steps(0, 1)
        qA, qB = qsteps[:len(qsteps) // 2], qsteps[len(qsteps) // 2:]
        kC, kD = ksteps[:len(ksteps) // 2], ksteps[len(ksteps) // 2:]
        for abc in zip(qA, qB, kC):
            for step in abc:
                step()
        for step in kD:
            step()

        # pair-0 scores woven with v waves and qk pair-1 waves, two
        # wave chains (= two av-psum slots) in flight at any time
        later = ([lambda j=j: emit_v_unit(j) for j in range(SQT)] +
                 emit_qk_wave_steps(1, 0) + emit_qk_wave_steps(1, 1))
        wi = 0
        blocks = ([(j, 0, j * 128, 1024) for j in range(8)] +
                  [(j, 1, max(j * 128, 1024), S) for j in range(SQT)])
        sc0 = [(j, half, a, b) for (j, half, lo, hi) in blocks
               for (a, b) in _sub512(lo, hi)]
        for n, blk in enumerate(sc0):
            emit_score_chunk(0, *blk)
            while wi * len(sc0) < (n + 1) * len(later):
                later[wi]()
                wi += 1

        # lookahead weave lists + segment schedule (cl = proj column base)
        proj012 = [(0, n, 1024) for n in range(NX // 128)] + \
                  [(1024, n, 512) for n in range(NX // 128)]
        proj3a = [(1536, n, 256) for n in range(NX // 128)]
        proj3b = [(1792, n, 256) for n in range(NX // 128)]

        def sc1(b):
            (j, half, lo, hi) = b
            return [lambda j=j, half=half, a=a, e=e:
                    emit_score_chunk(1, j, half, a, e)
                    for (a, e) in _sub512(lo, hi)]

        def pj(u):
            return lambda: proj_unit(*u)

        def run_pair(p, weave, woff, segments):
            total = sum((b + w) // 128 for (b, w, _) in segments)
            weave = list(weave)
            gstep, wi = 0, 0
            for (base, width, extra) in segments:
                if extra:
                    # segment-gated weave work (e.g. proj of a column range
                    # whose attT just completed)
                    weave.extend(extra)
                njs = (base + width) // 128
                avs = [avp.tile([65, width], F32, tag="av",
                                name=f"av{p}{base}{hl}") for hl in range(2)]
                for j in range(njs):
                    while (wi < len(weave) and gstep >= woff and
                           wi * (total - woff) < (gstep - woff + 1) * len(weave)):
                        weave[wi]()
                        wi += 1
                    for hl in range(2):
                        emit_av(p, hl, base, width, j, avs[hl])
                    gstep += 1
                emit_tail(p, base, avs[0], avs[1], width)
            while wi < len(weave):
                weave[wi]()
                wi += 1

        # pair-1's FIRST segment runs inside run_pair(0)'s weave: its two av
        # accumulators live in the two banks of ONE long-held sps slot (the
        # same pattern the wave chains use during sc0), soaking up PE idle
        # in the exp-bound middle and shrinking the serial back phase. Its
        # tail also completes attT[1][:, 0:512] early, so the proj weave
        # can start at gstep 8 instead of 12.
        p1s0_state = {}

        def p1seg0_step(j):
            def f():
                if "ps" not in p1s0_state:
                    p1s0_state["ps"] = psp.tile([128, 1024], F32, tag="sps",
                                                name="p1seg0")
                ps = p1s0_state["ps"]
                for hl in range(2):
                    emit_av(1, hl, 0, 512, j,
                            ps[0:65, hl * 512:hl * 512 + 512])
                if j == 3:
                    emit_tail(1, 0, ps[:, 0:512], ps[:, 512:1024], 512)
            return f

        sc1w = [f for b in blocks for f in sc1(b)]
        weave0 = (sc1w[:12] + [p1seg0_step(0)] + sc1w[12:16] +
                  [p1seg0_step(1)] + sc1w[16:20] + [p1seg0_step(2)] +
                  sc1w[20:24] + [p1seg0_step(3)] + sc1w[24:])
        seg512 = [(0, 512, None), (512, 512, None),
                  (1024, 512, None), (1536, 512, None)]
        run_pair(0, weave0, 0, seg512)
        # pair 1 (segment 0 already done above): split the last segment so
        # proj cols [1536:1792] can weave into the final segment's AV
        # instead of serializing after it
        seg_p1 = [(512, 512, None), (1024, 512, None),
                  (1536, 256, None), (1792, 256, [pj(u) for u in proj3a])]
        run_pair(1, [pj(u) for u in proj012], 8, seg_p1)

        # ---- projection (last column block) ----
        for u in proj3b:
            proj_unit(*u)
def build_nc():
    import concourse.bass as bass
    import concourse.mybir as mybir
    import concourse.tile as tile
    from concourse import bacc
    dt = mybir.dt
    F32, BF16 = dt.float32, dt.bfloat16

    nc = bacc.Bacc("TRN2", target_bir_lowering=False, debug=False, num_devices=8)
    xT = nc.dram_tensor("xT", [NX, S], BF16, kind="ExternalInput")
    wqk = nc.dram_tensor("wqk", [NX, 2 * HPC * D], BF16, kind="ExternalInput")
    wv = nc.dram_tensor("wv", [NX, HPC * D], BF16, kind="ExternalInput")
    wp = nc.dram_tensor("wp", [HPC * D, NX], BF16, kind="ExternalInput")
    bq = nc.dram_tensor("bq", [HPC * D, 1], F32, kind="ExternalInput")
    bk = nc.dram_tensor("bk", [HPC * D, 1], F32, kind="ExternalInput")
    maskT = nc.dram_tensor("maskT", [128, 128], BF16, kind="ExternalInput")
    outT = nc.dram_tensor("outT", [NX, S], BF16, kind="ExternalOutput")
    ri_dram = nc.dram_tensor("ri_scr", [HPC, S], BF16)
    tens = (xT, wqk, wv, wp, bq, bk, maskT, outT, ri_dram)

    with tile.TileContext(nc) as tc:
        _emit(nc, tc, bass, mybir, tens)
    nc.compile()
    return nc


def make_in_maps(x, w_attn, b_attn):
    bf = ml_dtypes.bfloat16
    maskT = np.triu(np.ones((128, 128), np.float32)).astype(bf)
    in_maps = []
    for core in range(8):
        b, g = divmod(core, 4)
        qs, ks, vs = 256 * g, NX + 256 * g, 2 * NX + 256 * g
        in_maps.append({
            "xT": np.ascontiguousarray(x[b].T).astype(bf),
            "wqk": np.ascontiguousarray(np.concatenate(
                [w_attn[:, qs:qs + 256], w_attn[:, ks:ks + 256]], axis=1)).astype(bf),
            "wv": np.ascontiguousarray(w_attn[:, vs:vs + 256]).astype(bf),
            "wp": None,  # filled by kernel() (needs w_proj)
            "bq": b_attn[qs:qs + 256].reshape(256, 1).astype(np.float32),
            "bk": b_attn[ks:ks + 256].reshape(256, 1).astype(np.float32),
            "maskT": maskT,
        })
    return in_maps


def kernel(**inputs):
    global _nc_cache
    x = np.asarray(inputs["x"], np.float32)
    w_attn = np.asarray(inputs["w_attn"], np.float32)
    b_attn = np.asarray(inputs["b_attn"], np.float32)
    w_proj = np.asarray(inputs["w_proj"], np.float32)
    b_proj = np.asarray(inputs["b_proj"], np.float32)

    bf = ml_dtypes.bfloat16
    in_maps = make_in_maps(x, w_attn, b_attn)
    for core in range(8):
        g = core % 4
        in_maps[core]["wp"] = np.ascontiguousarray(
            w_proj[256 * g:256 * (g + 1), :]).astype(bf)

    if _nc_cache is None:
        _nc_cache = build_nc()
    from concourse.bass_utils import run_bass_kernel_spmd
    res = run_bass_kernel_spmd(_nc_cache, in_maps, core_ids=list(range(8)))

    out = np.zeros((B, S, NX), np.float32)
    for core in range(8):
        out[core // 4] += res.results[core]["outT"].astype(np.float32).T
    bv = b_attn[2 * NX:3 * NX]
    out += (bv @ w_proj + b_proj)[None, None, :]
    return out

